# revision 23
# baseline (speedup 1.0000x reference)
"""Trainium2 Bass kernel for the LGP-instruction module (read -> op bank -> write).

Data-parallel over batch: core b computes x[b] (2048, 4096).
All HBM traffic is bf16 (x, weights, output) with fp32 PSUM accumulation:
~36 MB/core, DMA roofline ~95 us at ~390 GB/s/core.

Software-pipelined phase order per core (chunks of 512 T-columns):
  R0 O0 | R1 W0 O1 | R2 W1 O2 | R3 W2 O3 | W3
so the tensor queue never head-blocks on the op-bank ACT/STT chain, and
stores flow from ~t=25us on, overlapping the load stream.

  R(c): valuesT[C,512] = sum_vt rw[vt].T @ x_tile[vt]  (PSUM fp32)
  O(c): h_i = W_i.T @ valsT -> ACT f_i -> DVE weighted-accumulate
        (7 ops; the two linear bank ops are folded on host into one)
  W(c): out[128,V] blocks = accT.T @ wwT -> ACT/DVE drain -> SWDGE store

Host prep: read_w softmax, write_w*out_scale, x packed into
[chunk, block, 128, 4096] bf16 blocks (8KB DMA lines); output unpacked
bf16 -> fp32 on host.
"""
import sys
import numpy as np

if '/opt/trn_rl_repo' not in sys.path:
    sys.path.insert(0, '/opt/trn_rl_repo')

B, T, V, C = 8, 2048, 4096, 128
NCORES = 8
NV = V // 128     # 32 v-tiles
NTC = T // 512    # 4 T-chunks
NBLK = 4          # x load blocks per T-chunk
VB = NV // NBLK   # 8 v-tiles per block
NOP = 7           # op bank after folding the two linear ops

_CACHE = {}
LAST_RESULT = None


def _build(pre, post):
    from concourse import bass, bacc, tile, mybir
    f32, bf16 = mybir.dt.float32, mybir.dt.bfloat16
    AF = mybir.ActivationFunctionType
    ts = bass.ts
    # op order: [linear(init), gelu, tanh, sigmoid-as-tanh, relu, square, abs]
    # sigmoid is computed as 0.5*tanh(x/2)+0.5 (constant folded into the
    # linear op's bias) so all table-using activations share one table —
    # no per-chunk ACT table reloads. Table ops early so the accumulate
    # chain tail is ALU-only.
    FUNCS = [AF.Identity, AF.Gelu, AF.Tanh, AF.Tanh,
             AF.Relu, AF.Square, AF.Abs]

    nc = bacc.Bacc("TRN2", target_bir_lowering=False, debug=False,
                   num_devices=NCORES)
    xblk = nc.dram_tensor("xblk", [NTC, NBLK, 128, VB * 512], bf16,
                          kind="ExternalInput")
    rwp = nc.dram_tensor("rwp", [128, NV * C], bf16, kind="ExternalInput")
    wwT = nc.dram_tensor("wwT", [C, V], bf16, kind="ExternalInput")
    opw = nc.dram_tensor("opw", [C, NOP * C], bf16, kind="ExternalInput")
    opb = nc.dram_tensor("opb", [C, NOP], f32, kind="ExternalInput")
    out = nc.dram_tensor("out", [T, V], bf16, kind="ExternalOutput")

    with tile.TileContext(nc) as tc:
        with tc.tile_pool(name="const", bufs=1) as constp, \
             tc.tile_pool(name="xt", bufs=12) as xtp, \
             tc.tile_pool(name="vals_ps", bufs=2, space="PSUM") as vpsp, \
             tc.tile_pool(name="vals_sb", bufs=2) as vsbp, \
             tc.tile_pool(name="h_ps", bufs=2, space="PSUM") as hpsp, \
             tc.tile_pool(name="t_sb", bufs=3) as tp, \
             tc.tile_pool(name="acc", bufs=4) as accp, \
             tc.tile_pool(name="out_ps", bufs=2, space="PSUM") as opsp, \
             tc.tile_pool(name="out_sb", bufs=3) as osbp:

            # rw on the sync HWDGE queues (needed first, ahead of x);
            # the other params ride gpsimd SWDGE so they never delay x
            # loads, and are deferred below so they stay off the HBM
            # during the cold-ramp window when rw + the first x block
            # gate the first matmuls
            rw_t = constp.tile([128, NV, C], bf16)
            nc.sync.dma_start(rw_t[:], rwp.ap())
            wwT_t = constp.tile([C, V], bf16)
            opw_t = constp.tile([C, NOP, C], bf16)
            opb_t = constp.tile([C, NOP], f32)

            def load_params():
                nc.gpsimd.dma_start(opw_t[:], opw.ap())
                nc.gpsimd.dma_start(opb_t[:], opb.ap())
                nc.gpsimd.dma_start(wwT_t[:], wwT.ap())

            rd = {}

            def read_blk(tcn, blk):
                if blk == 0:
                    rd[tcn] = vpsp.tile([128, 512], f32, name="values")
                values = rd[tcn]
                xt = xtp.tile([128, VB, 512], bf16)
                if tcn == 0:
                    nc.sync.dma_start(xt[:, 0:VB // 2, :],
                                      xblk.ap()[tcn, blk, :, 0:VB * 256])
                    nc.sync.dma_start(xt[:, VB // 2:, :],
                                      xblk.ap()[tcn, blk, :, VB * 256:])
                else:
                    nc.sync.dma_start(xt[:], xblk.ap()[tcn, blk])
                for j in range(VB):
                    vt = blk * VB + j
                    nc.tensor.matmul(values[:], rw_t[:, vt, :], xt[:, j, :],
                                     start=(vt == 0), stop=(vt == NV - 1))
                if blk == NBLK - 1:
                    vals = vsbp.tile([128, 512], bf16)
                    nc.vector.tensor_copy(vals[:], values[:])
                    return vals

            def opbank(vals):
                acc = accp.tile([128, 512], f32)
                acc_bf = accp.tile([128, 512], bf16)
                for i in range(NOP):
                    h = hpsp.tile([128, 512], f32)
                    nc.tensor.matmul(h[:], opw_t[:, i, :], vals[:],
                                     start=True, stop=True)
                    if i == 0:
                        nc.scalar.activation(acc[:], h[:], FUNCS[0],
                                             bias=opb_t[:, 0:1], scale=pre[0])
                    else:
                        t = tp.tile([128, 512], bf16)
                        nc.scalar.activation(t[:], h[:], FUNCS[i],
                                             bias=opb_t[:, i:i + 1], scale=pre[i])
                        dst = acc_bf if i == NOP - 1 else acc
                        nc.vector.scalar_tensor_tensor(
                            dst[:], t[:], post[i], acc[:],
                            op0=mybir.AluOpType.mult, op1=mybir.AluOpType.add)
                return acc_bf

            def write_sub(tcn, sub, acc_bf):
                osb = osbp.tile([128, V], bf16)
                for half in range(4):
                    ops_ = opsp.tile([128, 1024], f32)
                    for q in range(2):
                        nc.tensor.matmul(
                            ops_[:, ts(q, 512)], acc_bf[:, ts(sub, 128)],
                            wwT_t[:, ts(half * 2 + q, 512)],
                            start=True, stop=True)
                    # 7 DVE / 9 ACT per chunk: ACT's copies are cheaper and
                    # DVE also carries the vals copy + 6 STTs
                    dve = half % 2 == 0 and not (sub == 3 and half == 2)
                    if dve:
                        nc.vector.tensor_copy(osb[:, ts(half, 1024)], ops_[:])
                    else:
                        nc.scalar.copy(osb[:, ts(half, 1024)], ops_[:])
                nc.gpsimd.dma_start(out.ap()[ts(tcn * 4 + sub, 128), :], osb[:])

            # one-stage software pipeline, period p emits [O(p-1)] then
            # interleaves W(p-1) sub-blocks with R(p) load-blocks:
            #   R0 | O0 R1/W0 | O1 R2/W1 | O2 R3/W2 | O3 W3
            # O(p-1) leads each period (its vals landed last period) so the
            # scalar queue starts activations immediately. W(p-1)'s first
            # sub is held until three R blocks have issued, by which time
            # the O(p-1) ACT/STT chain has produced acc_bf — so the tensor
            # queue never stalls on it. Interleaving W with R lets read
            # matmuls fill the tensor queue while W's drains catch up,
            # instead of drain backpressure stalling reads; only W3 runs
            # uncovered at the very end.
            vals = [None] * NTC
            accs = [None] * NTC
            for p in range(NTC + 1):
                if 1 <= p <= NTC and accs[p - 1] is None:
                    accs[p - 1] = opbank(vals[p - 1])
                if p == 0:
                    slots = [('r', 0), ('r', 1), ('r', 2), ('r', 3)]
                    read_blk(0, 0)
                    load_params()
                    slots = slots[1:]
                elif p < NTC:
                    slots = [('r', 0), ('r', 1), ('r', 2), ('w', 0),
                             ('r', 3), ('w', 1), ('w', 2), ('w', 3)]
                else:
                    slots = [('w', 0), ('w', 1), ('w', 2), ('w', 3)]
                for kind, k in slots:
                    if kind == 'r':
                        v = read_blk(p, k)
                        if v is not None:
                            vals[p] = v
                    elif kind == 'o':
                        accs[p] = opbank(vals[p])
                    else:
                        write_sub(p - 1, k, accs[p - 1])
    nc.compile()
    return nc


def _softmax(x, axis):
    x = np.asarray(x, np.float32)
    m = x.max(axis=axis, keepdims=True)
    e = np.exp(x - m)
    return e / e.sum(axis=axis, keepdims=True)


def _to_bf16(a):
    """Round-to-nearest-even fp32 -> bf16, fast numpy bit twiddle."""
    import ml_dtypes
    a = np.ascontiguousarray(a, np.float32)
    u = a.view(np.uint32)
    r = ((u >> 16) & 1) + np.uint32(0x7FFF)
    return ((u + r) >> 16).astype(np.uint16).view(ml_dtypes.bfloat16)


def kernel(x, basis, read_coeffs, write_coeffs, op_logits, op_weights,
           op_biases, out_scale):
    global LAST_RESULT
    from concourse.bass_utils import run_bass_kernel_spmd

    x = np.asarray(x, np.float32)
    basis = np.asarray(basis, np.float32)
    read_coeffs = np.asarray(read_coeffs, np.float32)
    write_coeffs = np.asarray(write_coeffs, np.float32)
    op_logits = np.asarray(op_logits, np.float32)
    op_weights = np.asarray(op_weights, np.float64)
    op_biases = np.asarray(op_biases, np.float64)
    out_scale = np.float32(out_scale)

    read_w = _softmax(basis @ read_coeffs.T, axis=0)               # (V, C)
    wwT = np.ascontiguousarray((basis @ write_coeffs.T).T) * out_scale  # (C, V)
    w = _softmax(op_logits, axis=0).astype(np.float64)

    # device op order: [linear(0&4 folded), gelu, tanh, sigmoid, relu, square, abs]
    # sigmoid op: w7*sigmoid(h+b7) = 0.5*w7*tanh(0.5*h + 0.5*b7) + 0.5*w7,
    # the 0.5*w7 constant rides on the linear op's bias
    W_lin = w[0] * op_weights[0] - w[4] * op_weights[4]
    b_lin = w[0] * op_biases[0] - w[4] * op_biases[4] + 0.5 * w[7]
    opw = np.stack([W_lin, op_weights[2], op_weights[6], op_weights[7],
                    op_weights[1], op_weights[3], op_weights[5]])
    pre = [1.0, 1.0, 1.0, 0.5, w[1], np.sqrt(w[3]), w[5]]
    post = [1.0, w[2], w[6], 0.5 * w[7], 1.0, 1.0, 1.0]
    pre = [float(v) for v in pre]
    post = [float(v) for v in post]
    opb = np.stack([b_lin, op_biases[2], op_biases[6],
                    0.5 * op_biases[7],
                    w[1] * op_biases[1], np.sqrt(w[3]) * op_biases[3],
                    w[5] * op_biases[5]], axis=1)  # (C, NOP)

    key = tuple(pre) + tuple(post)
    if key not in _CACHE:
        _CACHE[key] = _build(pre, post)
    nc = _CACHE[key]

    # pack params partition-major so DMA lines are long
    rwp = np.ascontiguousarray(
        _to_bf16(read_w).reshape(NV, 128, C).transpose(1, 0, 2).reshape(128, NV * C))
    opwp = np.ascontiguousarray(
        _to_bf16(opw.astype(np.float32)).transpose(1, 0, 2).reshape(128, NOP * C))
    shared = {
        "rwp": rwp,
        "wwT": _to_bf16(wwT),
        "opw": opwp,
        "opb": np.ascontiguousarray(opb, np.float32),
    }
    in_maps = []
    for b in range(B):
        m = dict(shared)
        # pack x[b] into [chunk, block, 128, VB*512] bf16 (8KB DMA lines)
        xb = _to_bf16(x[b])                       # (T, V)
        arr = xb.reshape(NTC, 512, NV, 128)       # (c, tt, vt, p)
        arr = arr.transpose(0, 2, 3, 1)           # (c, vt, p, tt)
        arr = arr.reshape(NTC, NBLK, VB, 128, 512).transpose(0, 1, 3, 2, 4)
        m["xblk"] = np.ascontiguousarray(arr.reshape(NTC, NBLK, 128, VB * 512))
        in_maps.append(m)

    res = run_bass_kernel_spmd(nc, in_maps, core_ids=list(range(NCORES)))
    LAST_RESULT = res
    out = np.empty((B, T, V), np.float32)
    for b in range(B):
        out[b] = np.asarray(res.results[b]["out"], np.float32)
    return out
